# revision 1
# baseline (speedup 1.0000x reference)
"""Causal self-attention (B=2, T=2048, C=1024, H=16, D=64) on 8 trn2 cores.

Sharding: tensor-parallel on heads — 2 heads per core. Each core computes
QKV projection for its 2 heads, causal softmax attention, and its heads'
slice of the output projection (a rank-128 partial sum of the full output).
The host pre-transposes x to [B, C, T], slices the weights per core, and
sums the 8 partial outputs (+ proj bias) at the end.

Device kernel layout notes (per core):
  - x^T chunks [128(C), T] stream from DRAM (host-transposed, contiguous
    DMA); QKV computed as W^T @ x^T giving q/k/v in [feat, tok] layout,
    which is what the attention matmuls want. All matmul inputs bf16,
    accumulation fp32.
  - S^T = kT-slice.T @ qT tile -> [128(k), Q] PSUM; exp on ScalarE with
    the 1/sqrt(D) scale fused; causality via subtile skipping, a
    diagonal-aligned exp range, and one [128,128] triangle mask multiply.
  - O^T accumulates as (V|1)-chunk.T @ P^T; the ones column makes row 64
    the softmax denominator for free. Normalization: fast reciprocal,
    GpSimd partition-broadcast, one multiply into a [128, Q] tile with
    both heads stacked.
  - Projection: single K=128 matmul per output tile (heads contracted
    together against the raw [128, C] proj_w slice).
  - The attention inner loop ping-pongs PE<->ACT, which leaves the PE
    sparse and HAM-throttled at 1.2 GHz. Dense independent PE work (next
    batch's QKV, V transposes, previous q-tile's projection) is emitted
    through a filler queue, one item per chunk, to keep the PE streaming.
"""

from collections import deque

import numpy as np

import concourse.bass as bass
import concourse.tile as tile
from concourse import bacc, mybir
from concourse.bass_utils import run_bass_kernel_spmd

dt = mybir.dt
AF = mybir.ActivationFunctionType

B, T, C, H, D = 2, 2048, 1024, 16, 64
NCORES = 8
HPC = H // NCORES          # heads per core = 2
QT = 1024                  # q-tile (columns of S^T/O^T psum tiles)
KC = 128                   # k chunk (partition dim of S^T)
SUB = 512                  # psum bank subtile (fp32)
SCALE = 1.0 / 8.0          # 1/sqrt(D)

_CACHE = {}


def _emit(tc):
    from contextlib import ExitStack
    with ExitStack() as ctx:
        _emit_body(tc, ctx)


def _emit_body(tc, ctx):
    nc = tc.nc
    f32, bf16 = dt.float32, dt.bfloat16

    xT = nc.dram_tensor("xT", [B, C, T], bf16, kind="ExternalInput").ap()
    wqkv = nc.dram_tensor("wqkv", [C, 384], bf16, kind="ExternalInput").ap()
    bqkv = nc.dram_tensor("bqkv", [128, 3], f32, kind="ExternalInput").ap()
    wp = nc.dram_tensor("wp", [128, C], bf16, kind="ExternalInput").ap()
    tri = nc.dram_tensor("tri", [128, 128], bf16, kind="ExternalInput").ap()
    ident = nc.dram_tensor("ident", [128, 128], bf16, kind="ExternalInput").ap()
    outp = nc.dram_tensor("outp", [B, T, C], f32, kind="ExternalOutput").ap()

    consts = ctx.enter_context(tc.tile_pool(name="consts", bufs=1))
    xpool = ctx.enter_context(tc.tile_pool(name="xpool", bufs=2))
    qkvpool = ctx.enter_context(tc.tile_pool(name="qkvpool", bufs=6))
    vtmpool = ctx.enter_context(tc.tile_pool(name="vtmpool", bufs=2))
    ptpool = ctx.enter_context(tc.tile_pool(name="ptpool", bufs=8))
    unormp = ctx.enter_context(tc.tile_pool(name="unormp", bufs=3))
    rows = ctx.enter_context(tc.tile_pool(name="rows", bufs=4))
    outsb = ctx.enter_context(tc.tile_pool(name="outsb", bufs=8))
    stp = ctx.enter_context(tc.tile_pool(name="stp", bufs=2, space="PSUM"))
    otp = ctx.enter_context(tc.tile_pool(name="otp", bufs=1, space="PSUM"))
    miscp = ctx.enter_context(tc.tile_pool(name="miscp", bufs=2, space="PSUM"))

    # constants / weights resident in SBUF
    w_sb = consts.tile([128, 8, 384], bf16, tag="w")
    nc.sync.dma_start(out=w_sb, in_=wqkv.rearrange("(k p) f -> p k f", p=128))
    b_sb = consts.tile([128, 3], f32, tag="b")
    nc.sync.dma_start(out=b_sb, in_=bqkv)
    wp_sb = consts.tile([128, C], bf16, tag="wp")
    nc.sync.dma_start(out=wp_sb, in_=wp)
    tri_sb = consts.tile([128, 128], bf16, tag="tri")
    nc.sync.dma_start(out=tri_sb, in_=tri)
    id_sb = consts.tile([128, 128], bf16, tag="id")
    nc.sync.dma_start(out=id_sb, in_=ident)

    # x^T for both batches (sync queue, ahead of output stores)
    xps = []
    for b in range(B):
        xp = xpool.tile([128, 8, T], bf16, tag="xp", name=f"xp{b}")
        xsrc = xT[b].rearrange("(j p) t -> p j t", p=128)
        for tg in range(T // SUB):
            t0 = tg * SUB
            nc.sync.dma_start(out=xp[:, :, t0:t0 + SUB],
                              in_=xsrc[:, :, t0:t0 + SUB])
        xps.append(xp)

    filler = deque()

    def pop_filler():
        if filler:
            filler.popleft()()

    def make_qkv(b):
        """qkvT tiles + one thunk per (m, token-group): an 8-MM dense chain."""
        dsts = [qkvpool.tile([128, T], bf16, tag="qkv", name=f"qkv{b}_{m}")
                for m in range(3)]
        thunks = []
        for tg in range(T // 1024):
            for m in range(3):
                def th(m=m, tg=tg):
                    pgs = [miscp.tile([128, SUB], f32, tag="misc",
                                      name=f"pg{n}") for n in range(2)]
                    for kc in range(8):
                        for n in range(2):
                            t0 = tg * 1024 + n * SUB
                            nc.tensor.matmul(
                                pgs[n][:, :],
                                w_sb[:, kc, 128 * m:128 * m + 128],
                                xps[b][:, kc, t0:t0 + SUB],
                                start=(kc == 0), stop=(kc == 7),
                            )
                    for n in range(2):
                        t0 = tg * 1024 + n * SUB
                        nc.scalar.activation(
                            dsts[m][:, t0:t0 + SUB], pgs[n][:, :],
                            AF.Identity, bias=b_sb[:, m:m + 1])
                thunks.append(th)
        return dsts, thunks

    def make_vt(b, vT_t):
        """V to token-major [128, 16, 2*65] with ones columns; 9 thunks."""
        vt = vtmpool.tile([128, 16, HPC * 65], bf16, tag="vtm", name=f"vt{b}")

        def th0():
            nc.vector.memset(
                vt.rearrange("p k (h c) -> p k h c", h=HPC)[:, :, :, 64:65],
                1.0)
        thunks = [th0]
        for j0 in range(0, T // 128, 2):
            def th(j0=j0):
                for j in (j0, j0 + 1):
                    tp = miscp.tile([128, 128], bf16, tag="misc", name="tp")
                    nc.tensor.transpose(
                        tp[:, :], vT_t[:, 128 * j:128 * j + 128], id_sb[:, :])
                    nc.vector.tensor_copy(
                        out=vt[:, j, :].rearrange(
                            "p (h c) -> p h c", h=HPC)[:, :, 0:64],
                        in_=tp.rearrange("p (h c) -> p h c", h=HPC),
                    )
            thunks.append(th)
        return vt, thunks

    def make_proj(b, q0, un):
        """Projection of one q-tile: 16 single-matmul thunks."""
        thunks = []
        for ts in range(QT // 128):
            for ct in range(C // SUB):
                def th(ts=ts, ct=ct):
                    a0 = q0 + ts * 128
                    pp = miscp.tile([128, SUB], f32, tag="misc", name="pp")
                    nc.tensor.matmul(
                        pp[:, :],
                        un[:, ts * 128:(ts + 1) * 128],
                        wp_sb[:, ct * SUB:(ct + 1) * SUB],
                        start=True, stop=True,
                    )
                    ob = outsb.tile([128, SUB], f32, tag="osb")
                    nc.vector.tensor_copy(ob[:, :], pp[:, :])
                    nc.sync.dma_start(
                        out=outp[b, a0:a0 + 128, ct * SUB:(ct + 1) * SUB],
                        in_=ob[:, :])
                thunks.append(th)
        return thunks

    # batch 0 front work runs densely right away
    qkv0, th0 = make_qkv(0)
    for th in th0:
        th()
    vt0, vth0 = make_vt(0, qkv0[2])
    for th in vth0:
        th()

    qkv_t, vt_t = {0: qkv0}, {0: vt0}

    for b in range(B):
        if b == 0:
            # queue batch 1 front work as attention filler
            qkv1, th1 = make_qkv(1)
            vt1, vth1 = make_vt(1, qkv1[2])
            filler.extend(th1)
            filler.extend(vth1)
            qkv_t[1], vt_t[1] = qkv1, vt1
        qT_t, kT_t, vT_t = qkv_t[b]
        vt = vt_t[b]

        for qt in range(T // QT):
            q0 = qt * QT
            nkc = (q0 + QT) // KC
            un = unormp.tile([128, QT], bf16, tag="un", name=f"un{b}{qt}")
            for h in range(HPC):
                qT_h = qT_t[64 * h:64 * h + 64, :]
                kT_h = kT_t[64 * h:64 * h + 64, :]
                ot = otp.tile([65, QT], f32, tag="ot")

                def emit_o(kc, pt_):
                    ls = max(0, kc * KC - q0)
                    diag = kc * KC >= q0
                    for n in range(QT // SUB):
                        s0 = max(n * SUB, ls)
                        if s0 >= (n + 1) * SUB:
                            continue
                        if diag and s0 == ls:
                            s0 = ls + 128  # masked strip emitted separately
                            if s0 >= (n + 1) * SUB:
                                continue
                        last_kc = (q0 + (n + 1) * SUB) // KC - 1
                        nc.tensor.matmul(
                            ot[:, s0:(n + 1) * SUB],
                            vt[:, kc, 65 * h:65 * h + 65],
                            pt_[:, s0:(n + 1) * SUB],
                            start=(kc == 0), stop=(kc == last_kc),
                        )
                    if diag:
                        # region already started by kc=0's full-subtile MM
                        n0 = ls // SUB
                        last_kc = (q0 + (n0 + 1) * SUB) // KC - 1
                        nc.tensor.matmul(
                            ot[:, ls:ls + 128],
                            vt[:, kc, 65 * h:65 * h + 65],
                            pt_[:, ls:ls + 128],
                            start=False, stop=(kc == last_kc),
                        )

                for kc in range(nkc):
                    k0 = kc * KC
                    ls = max(0, k0 - q0)
                    st = stp.tile([128, QT], f32, tag="st")
                    pt_ = ptpool.tile([128, QT], bf16, tag="pt")
                    for n in range(QT // SUB):
                        s0 = max(n * SUB, ls)
                        if s0 >= (n + 1) * SUB:
                            continue
                        nc.tensor.matmul(
                            st[:, s0:(n + 1) * SUB],
                            kT_h[:, k0:k0 + KC],
                            qT_h[:, q0 + s0:q0 + (n + 1) * SUB],
                            start=True, stop=True,
                        )
                    nc.scalar.activation(
                        pt_[:, ls:QT], st[:, ls:QT], AF.Exp, scale=SCALE)
                    if k0 >= q0:  # diagonal chunk: zero invalid triangle
                        nc.vector.tensor_mul(
                            pt_[:, ls:ls + 128], pt_[:, ls:ls + 128],
                            tri_sb[:, :])
                    emit_o(kc, pt_)
                    if not (b == 0 and qt == 0 and kc % 2 == 0):
                        pop_filler()

                # normalize into this head's half of un
                se = rows.tile([1, QT], f32, tag="se", name=f"se{h}")
                nc.vector.tensor_copy(se[:, :], ot[64:65, :])
                rc = rows.tile([1, QT], f32, tag="rc", name=f"rc{h}")
                nc.vector.reciprocal_approx_fast(rc[:, :], se[:, :])
                rb = rows.tile([64, QT], f32, tag="rb", name=f"rb{h}")
                nc.gpsimd.partition_broadcast(rb[:, :], rc[:, :])
                nc.vector.tensor_mul(
                    un[64 * h:64 * h + 64, :], ot[0:64, :], rb[:, :])
            filler.extend(make_proj(b, q0, un))

    while filler:
        pop_filler()


def build():
    if "nc" in _CACHE:
        return _CACHE["nc"]
    nc = bacc.Bacc("TRN2", target_bir_lowering=False, debug=False,
                   num_devices=NCORES)
    with tile.TileContext(nc) as tc:
        _emit(tc)
    nc.compile()
    _CACHE["nc"] = nc
    return nc


def make_in_maps(x, qkv_w, qkv_b, proj_w):
    import ml_dtypes
    bf16 = ml_dtypes.bfloat16
    x = np.asarray(x, dtype=np.float32)
    qkv_w = np.asarray(qkv_w, dtype=np.float32)
    qkv_b = np.asarray(qkv_b, dtype=np.float32)
    proj_w = np.asarray(proj_w, dtype=np.float32)

    xT = np.ascontiguousarray(x.transpose(0, 2, 1)).astype(bf16)
    tri = (np.arange(128)[None, :] >= np.arange(128)[:, None]).astype(bf16)
    ident = np.eye(128, dtype=bf16)

    in_maps = []
    for c in range(NCORES):
        s = 64 * HPC * c  # first feature row of this core's heads
        wq = qkv_w[:, s:s + 128]
        wk = qkv_w[:, C + s:C + s + 128]
        wv = qkv_w[:, 2 * C + s:2 * C + s + 128]
        wqkv_c = np.ascontiguousarray(
            np.concatenate([wq, wk, wv], axis=1)).astype(bf16)
        bqkv_c = np.ascontiguousarray(np.stack(
            [qkv_b[s:s + 128], qkv_b[C + s:C + s + 128],
             qkv_b[2 * C + s:2 * C + s + 128]], axis=1))
        wp_c = np.ascontiguousarray(proj_w[s:s + 128, :]).astype(bf16)
        in_maps.append({
            "xT": xT, "wqkv": wqkv_c, "bqkv": bqkv_c, "wp": wp_c,
            "tri": tri, "ident": ident,
        })
    return in_maps


def kernel(x, qkv_w, qkv_b, proj_w, proj_b, _trace=False):
    nc = build()
    in_maps = make_in_maps(x, qkv_w, qkv_b, proj_w)
    res = run_bass_kernel_spmd(nc, in_maps, core_ids=list(range(NCORES)),
                               trace=_trace)
    acc = np.zeros((B, T, C), dtype=np.float64)
    for c in range(NCORES):
        acc += res.results[c]["outp"].astype(np.float64)
    acc += np.asarray(proj_b, dtype=np.float64)
    out = acc.astype(np.float32)
    _CACHE["last_results"] = res
    return out



# revision 11
# speedup vs baseline: 1.0016x; 1.0016x over previous
"""Causal self-attention (B=2, T=2048, C=1024, H=16, D=64) on 8 trn2 cores.

Sharding: tensor-parallel on heads - 2 heads per core. Each core computes
QKV projection for its 2 heads, causal softmax attention, and its heads'
slice of the output projection (a rank-128 partial sum of the full output).
The host pre-transposes x to [B, C, T], slices the weights per core, and
sums the 8 partial outputs (+ proj bias) at the end. Partials are stored
fp16 to halve the output DMA traffic.

Key device-kernel structure (per core), v2:
  - q-tiles of 512 columns (QT=512). Per k-chunk (128 tokens) the two
    heads' S^T matmuls are emitted back-to-back: head0's kT/qT live on
    partitions 0-63, head1's on 64-127, so the PE runs them CONCURRENTLY
    in different 64-row groups of the systolic array (row tiling),
    halving S time vs. serial heads. Each writes its own PSUM bank.
  - exp on ScalarE per (chunk, head) with the 1/sqrt(D) scale fused;
    causality via chunk skipping plus one [128,128] triangle mask
    multiply per diagonal chunk (VectorE).
  - O^T accumulates per head as (V|1)-chunk.T @ P^T; the ones column
    makes row 64 the softmax denominator for free. One matmul per
    (chunk, head), accumulating in a [65, 512] PSUM bank per head.
  - Normalization: fast reciprocal (DVE), GpSimd partition-broadcast,
    one multiply into un [128, 512] with both heads stacked.
  - QKV psum->sbuf eviction with bias moved to VectorE (tensor_scalar),
    freeing ScalarE for exp.
  - Projection: single K=128 matmul per [128 tok, 512] output tile;
    psum->sbuf fp16 conversion alternates Scalar/Vector engines.
  - PE kept dense via a filler queue (later QKV token-groups, V
    transposes, projection of finished q-tiles) popped inside the
    attention chunk loop.

PSUM budget (8 banks x 2KB): S tiles 4 x [128,512]f32, O tiles
2 x [65,512]f32, misc (QKV/proj/transpose) 2 x [128,512]f32.
"""

from collections import deque

import numpy as np

import concourse.bass as bass
import concourse.tile as tile
from concourse import bacc, mybir
from concourse.bass_utils import run_bass_kernel_spmd

dt = mybir.dt
AF = mybir.ActivationFunctionType

B, T, C, H, D = 2, 2048, 1024, 16, 64
NCORES = 8
HPC = H // NCORES          # heads per core = 2
QT = 512                   # q-tile (columns of S^T/O^T psum tiles)
KC = 128                   # k chunk (partition dim of S^T)
NQT = T // QT              # q-tiles per batch = 4
SCALE = 1.0 / 8.0          # 1/sqrt(D)

_CACHE = {}


def _emit(tc):
    from contextlib import ExitStack
    with ExitStack() as ctx:
        _emit_body(tc, ctx)


def _emit_body(tc, ctx):
    nc = tc.nc
    f32, bf16, f16 = dt.float32, dt.bfloat16, dt.float16

    import os
    debug = bool(int(os.environ.get("K_DEBUG", "0")))
    xT = nc.dram_tensor("xT", [B, C, T], bf16, kind="ExternalInput").ap()
    wqkv = nc.dram_tensor("wqkv", [C, 384], bf16, kind="ExternalInput").ap()
    bqkv = nc.dram_tensor("bqkv", [128, 3], f32, kind="ExternalInput").ap()
    wp = nc.dram_tensor("wp", [128, C], bf16, kind="ExternalInput").ap()
    tri = nc.dram_tensor("tri", [128, 128], bf16, kind="ExternalInput").ap()
    ident = nc.dram_tensor("ident", [128, 128], bf16, kind="ExternalInput").ap()
    outp = nc.dram_tensor("outp", [B, T, C], f16, kind="ExternalOutput").ap()
    if debug:
        dbg_qkv = nc.dram_tensor("dbg_qkv", [3, 128, T], bf16,
                                 kind="ExternalOutput").ap()
        dbg_vt = nc.dram_tensor("dbg_vt", [128, 16, HPC * 65], bf16,
                                kind="ExternalOutput").ap()
        dbg_pt = nc.dram_tensor("dbg_pt", [HPC, 128, QT], bf16,
                                kind="ExternalOutput").ap()
        dbg_un = nc.dram_tensor("dbg_un", [128, QT], bf16,
                                kind="ExternalOutput").ap()
        dbg_ot = nc.dram_tensor("dbg_ot", [HPC, 65, QT], f32,
                                kind="ExternalOutput").ap()

    consts = ctx.enter_context(tc.tile_pool(name="consts", bufs=1))
    xpool = ctx.enter_context(tc.tile_pool(name="xpool", bufs=2))
    qkvpool = ctx.enter_context(tc.tile_pool(name="qkvpool", bufs=6))
    vtmpool = ctx.enter_context(tc.tile_pool(name="vtmpool", bufs=2))
    ptpool = ctx.enter_context(tc.tile_pool(name="ptpool", bufs=4))
    unormp = ctx.enter_context(tc.tile_pool(name="unormp", bufs=2))
    rows = ctx.enter_context(tc.tile_pool(name="rows", bufs=6))
    outsb = ctx.enter_context(tc.tile_pool(name="outsb", bufs=8))
    stp = ctx.enter_context(tc.tile_pool(name="stp", bufs=4, space="PSUM"))
    otp = ctx.enter_context(tc.tile_pool(name="otp", bufs=2, space="PSUM"))
    miscp = ctx.enter_context(tc.tile_pool(name="miscp", bufs=2, space="PSUM"))

    # constants / weights resident in SBUF
    w_sb = consts.tile([128, 8, 384], bf16, tag="w")
    nc.sync.dma_start(out=w_sb, in_=wqkv.rearrange("(k p) f -> p k f", p=128))
    b_sb = consts.tile([128, 3], f32, tag="b")
    nc.sync.dma_start(out=b_sb, in_=bqkv)
    wp_sb = consts.tile([128, C], bf16, tag="wp")
    nc.sync.dma_start(out=wp_sb, in_=wp)
    tri_sb = consts.tile([128, 128], bf16, tag="tri")
    nc.sync.dma_start(out=tri_sb, in_=tri)
    id_sb = consts.tile([128, 128], bf16, tag="id")
    nc.sync.dma_start(out=id_sb, in_=ident)

    # x^T for both batches, in 512-token groups so compute starts early
    xps = []
    for b in range(B):
        xp = xpool.tile([128, 8, T], bf16, tag="xp", name=f"xp{b}")
        xsrc = xT[b].rearrange("(j p) t -> p j t", p=128)
        for tg in range(NQT):
            t0 = tg * QT
            nc.sync.dma_start(out=xp[:, :, t0:t0 + QT],
                              in_=xsrc[:, :, t0:t0 + QT])
        xps.append(xp)

    filler = deque()

    def pop_filler(n=1):
        for _ in range(n):
            if filler:
                filler.popleft()()

    def make_qkv(b):
        """qkvT tiles + one thunk per (tg, m): an 8-MM chain + DVE evict."""
        dsts = [qkvpool.tile([128, T], bf16, tag="qkv", name=f"qkv{b}_{m}")
                for m in range(3)]
        thunks = []
        for tg in range(NQT):
            for m in range(3):
                def th(m=m, tg=tg):
                    t0 = tg * QT
                    pg = miscp.tile([128, QT], f32, tag="misc", name="pg")
                    for kc in range(8):
                        nc.tensor.matmul(
                            pg[:, :],
                            w_sb[:, kc, 128 * m:128 * m + 128],
                            xps[b][:, kc, t0:t0 + QT],
                            start=(kc == 0), stop=(kc == 7),
                        )
                    nc.scalar.activation(
                        dsts[m][:, t0:t0 + QT], pg[:, :],
                        AF.Identity, bias=b_sb[:, m:m + 1])
                thunks.append(th)
        return dsts, thunks

    def make_vt(b, vT_t):
        """V to token-major [128, 16, 2*65] with ones columns; 9 thunks."""
        vt = vtmpool.tile([128, 16, HPC * 65], bf16, tag="vtm", name=f"vt{b}")

        def th0():
            nc.vector.memset(
                vt.rearrange("p k (h c) -> p k h c", h=HPC)[:, :, :, 64:65],
                1.0)
        thunks = [th0]
        for j0 in range(0, T // 128, 2):
            def th(j0=j0):
                for j in (j0, j0 + 1):
                    tp = miscp.tile([128, 128], bf16, tag="misc", name="tp")
                    nc.tensor.transpose(
                        tp[:, :], vT_t[:, 128 * j:128 * j + 128], id_sb[:, :])
                    nc.vector.tensor_copy(
                        out=vt[:, j, :].rearrange(
                            "p (h c) -> p h c", h=HPC)[:, :, 0:64],
                        in_=tp.rearrange("p (h c) -> p h c", h=HPC),
                    )
            thunks.append(th)
        return vt, thunks

    def make_proj(b, q0, un):
        """Projection of one q-tile: 8 thunks (1 MM + evict + DMA each)."""
        thunks = []
        for ts in range(QT // 128):
            for ct in range(C // 512):
                def th(ts=ts, ct=ct):
                    a0 = q0 + ts * 128
                    pp = miscp.tile([128, 512], f32, tag="misc", name="pp")
                    nc.tensor.matmul(
                        pp[:, :],
                        un[:, ts * 128:(ts + 1) * 128],
                        wp_sb[:, ct * 512:(ct + 1) * 512],
                        start=True, stop=True,
                    )
                    ob = outsb.tile([128, 512], f16, tag="osb")
                    nc.vector.tensor_copy(ob[:, :], pp[:, :])
                    nc.sync.dma_start(
                        out=outp[b, a0:a0 + 128, ct * 512:(ct + 1) * 512],
                        in_=ob[:, :])
                thunks.append(th)
        return thunks

    # front work: batch-0 QKV for tokens 0..511 and V-chunks 0..3 run
    # densely right away; the rest becomes attention filler.
    qkv0, th_qkv0 = make_qkv(0)
    vt0, th_vt0 = make_vt(0, qkv0[2])
    for i in range(3):       # tg0 x (q, k, v)
        th_qkv0[i]()
    th_vt0[0]()              # ones memset
    th_vt0[1]()              # chunks 0-1
    th_vt0[2]()              # chunks 2-3
    filler.extend(th_qkv0[3:])
    filler.extend(th_vt0[3:])

    qkv1, th_qkv1 = make_qkv(1)
    vt1, th_vt1 = make_vt(1, qkv1[2])
    filler.extend(th_qkv1)
    filler.extend(th_vt1)

    qkv_t, vt_t = {0: qkv0, 1: qkv1}, {0: vt0, 1: vt1}

    for b in range(B):
        qT_t, kT_t, vT_t = qkv_t[b]
        vt = vt_t[b]

        for qt in range(NQT):
            q0 = qt * QT
            nkc = (q0 + QT) // KC
            un = unormp.tile([128, QT], bf16, tag="un", name=f"un{b}{qt}")
            ots = [otp.tile([65, QT], f32, tag="ot", name=f"ot{h}")
                   for h in range(HPC)]

            def emit_o(kc, pts):
                ls = max(0, kc * KC - q0)
                for h in range(HPC):
                    nc.tensor.matmul(
                        ots[h][:, ls:QT],
                        vt[:, kc, 65 * h:65 * h + 65],
                        pts[h][:, ls:QT],
                        start=(kc == 0), stop=(kc == nkc - 1),
                    )

            prev = None
            for kc in range(nkc):
                k0 = kc * KC
                ls = max(0, k0 - q0)
                sts = [stp.tile([128, QT], f32, tag="st", name=f"st{h}")
                       for h in range(HPC)]
                pts = [ptpool.tile([128, QT], bf16, tag="pt", name=f"pt{h}")
                       for h in range(HPC)]
                # both heads' S matmuls back-to-back -> concurrent row
                # groups (h0: rows 0-63, h1: rows 64-127)
                for h in range(HPC):
                    nc.tensor.matmul(
                        sts[h][:, ls:QT],
                        kT_t[64 * h:64 * h + 64, k0:k0 + KC],
                        qT_t[64 * h:64 * h + 64, q0 + ls:q0 + QT],
                        start=True, stop=True,
                    )
                for h in range(HPC):
                    nc.scalar.activation(
                        pts[h][:, ls:QT], sts[h][:, ls:QT], AF.Exp,
                        scale=SCALE)
                if k0 >= q0:  # diagonal chunk: zero invalid triangle
                    for h in range(HPC):
                        nc.vector.tensor_mul(
                            pts[h][:, ls:ls + 128], pts[h][:, ls:ls + 128],
                            tri_sb[:, :])
                if debug and b == 0 and qt == 0 and kc == 1:
                    for h in range(HPC):
                        nc.sync.dma_start(out=dbg_pt[h], in_=pts[h])
                if prev is not None:
                    emit_o(*prev)
                pop_filler()
                prev = (kc, pts)
            emit_o(*prev)

            # normalize both heads into un
            for h in range(HPC):
                if debug and b == 0 and qt == 0:
                    osb_d = rows.tile([65, QT], f32, tag="osbd",
                                      name=f"osbd{h}")
                    nc.vector.tensor_copy(osb_d[:, :], ots[h][:, :])
                    nc.sync.dma_start(out=dbg_ot[h], in_=osb_d)
                se = rows.tile([1, QT], f32, tag="se", name=f"se{h}")
                nc.vector.tensor_copy(se[:, :], ots[h][64:65, :])
                rc = rows.tile([1, QT], f32, tag="rc", name=f"rc{h}")
                nc.vector.reciprocal_approx_fast(rc[:, :], se[:, :])
                rb = rows.tile([64, QT], f32, tag="rb", name=f"rb{h}")
                nc.gpsimd.partition_broadcast(rb[:, :], rc[:, :])
                nc.vector.tensor_mul(
                    un[64 * h:64 * h + 64, :], ots[h][0:64, :], rb[:, :])
            if debug and b == 0 and qt == 0:
                nc.sync.dma_start(out=dbg_un, in_=un)
            filler.extend(make_proj(b, q0, un))
            pop_filler(3)

    while filler:
        pop_filler()

    if debug:
        for m in range(3):
            nc.sync.dma_start(out=dbg_qkv[m], in_=qkv0[m])
        nc.sync.dma_start(out=dbg_vt, in_=vt0)


def build():
    if "nc" in _CACHE:
        return _CACHE["nc"]
    nc = bacc.Bacc("TRN2", target_bir_lowering=False, debug=False,
                   num_devices=NCORES)
    with tile.TileContext(nc) as tc:
        _emit(tc)
    nc.compile()
    _CACHE["nc"] = nc
    return nc


def make_in_maps(x, qkv_w, qkv_b, proj_w):
    import ml_dtypes
    bf16 = ml_dtypes.bfloat16
    x = np.asarray(x, dtype=np.float32)
    qkv_w = np.asarray(qkv_w, dtype=np.float32)
    qkv_b = np.asarray(qkv_b, dtype=np.float32)
    proj_w = np.asarray(proj_w, dtype=np.float32)

    xT = np.ascontiguousarray(x.transpose(0, 2, 1)).astype(bf16)
    tri = (np.arange(128)[None, :] >= np.arange(128)[:, None]).astype(bf16)
    ident = np.eye(128, dtype=bf16)

    in_maps = []
    for c in range(NCORES):
        s = 64 * HPC * c  # first feature row of this core's heads
        wq = qkv_w[:, s:s + 128]
        wk = qkv_w[:, C + s:C + s + 128]
        wv = qkv_w[:, 2 * C + s:2 * C + s + 128]
        wqkv_c = np.ascontiguousarray(
            np.concatenate([wq, wk, wv], axis=1)).astype(bf16)
        bqkv_c = np.ascontiguousarray(np.stack(
            [qkv_b[s:s + 128], qkv_b[C + s:C + s + 128],
             qkv_b[2 * C + s:2 * C + s + 128]], axis=1))
        wp_c = np.ascontiguousarray(proj_w[s:s + 128, :]).astype(bf16)
        in_maps.append({
            "xT": xT, "wqkv": wqkv_c, "bqkv": bqkv_c, "wp": wp_c,
            "tri": tri, "ident": ident,
        })
    return in_maps


def kernel(x, qkv_w, qkv_b, proj_w, proj_b, _trace=False):
    nc = build()
    in_maps = make_in_maps(x, qkv_w, qkv_b, proj_w)
    res = run_bass_kernel_spmd(nc, in_maps, core_ids=list(range(NCORES)),
                               trace=_trace)
    acc = np.zeros((B, T, C), dtype=np.float64)
    for c in range(NCORES):
        acc += np.asarray(res.results[c]["outp"], dtype=np.float64)
    acc += np.asarray(proj_b, dtype=np.float64)
    out = acc.astype(np.float32)
    _CACHE["last_results"] = res
    return out


# revision 19
# speedup vs baseline: 1.0718x; 1.0700x over previous
"""Causal self-attention (B=2, T=2048, C=1024, H=16, D=64) on 8 trn2 cores.

Sharding: tensor-parallel on heads - 2 heads per core. Each core computes
QKV projection for its 2 heads, causal softmax attention, and its heads'
slice of the output projection (a rank-128 partial sum of the full output).
The host pre-transposes x to [B, C, T], slices the weights per core, and
sums the 8 partial outputs (+ proj bias) at the end. Partials are stored
fp16 to halve the output DMA traffic.

Key device-kernel structure (per core), v2:
  - q-tiles of 512 columns (QT=512). Per k-chunk (128 tokens) the two
    heads' S^T matmuls are emitted back-to-back: head0's kT/qT live on
    partitions 0-63, head1's on 64-127, so the PE runs them CONCURRENTLY
    in different 64-row groups of the systolic array (row tiling),
    halving S time vs. serial heads. Each writes its own PSUM bank.
  - exp on ScalarE per (chunk, head) with the 1/sqrt(D) scale fused;
    causality via chunk skipping plus one [128,128] triangle mask
    multiply per diagonal chunk (VectorE).
  - O^T accumulates per head as (V|1)-chunk.T @ P^T; the ones column
    makes row 64 the softmax denominator for free. One matmul per
    (chunk, head), accumulating in a [65, 512] PSUM bank per head.
  - Normalization: fast reciprocal (DVE), GpSimd partition-broadcast,
    one multiply into un [128, 512] with both heads stacked.
  - QKV psum->sbuf eviction with bias moved to VectorE (tensor_scalar),
    freeing ScalarE for exp.
  - Projection: single K=128 matmul per [128 tok, 512] output tile;
    psum->sbuf fp16 conversion alternates Scalar/Vector engines.
  - PE kept dense via a filler queue (later QKV token-groups, V
    transposes, projection of finished q-tiles) popped inside the
    attention chunk loop.

PSUM budget (8 banks x 2KB): S tiles 4 x [128,512]f32, O tiles
2 x [65,512]f32, misc (QKV/proj/transpose) 2 x [128,512]f32.
"""

from collections import deque

import numpy as np

import concourse.bass as bass
import concourse.tile as tile
from concourse import bacc, mybir
from concourse.bass_utils import run_bass_kernel_spmd

dt = mybir.dt
AF = mybir.ActivationFunctionType

B, T, C, H, D = 2, 2048, 1024, 16, 64
NCORES = 8
HPC = H // NCORES          # heads per core = 2
QT = 512                   # q-tile (columns of S^T/O^T psum tiles)
KC = 128                   # k chunk (partition dim of S^T)
NQT = T // QT              # q-tiles per batch = 4
SCALE = 1.0 / 8.0          # 1/sqrt(D)

_CACHE = {}


def _emit(tc):
    from contextlib import ExitStack
    with ExitStack() as ctx:
        _emit_body(tc, ctx)


def _emit_body(tc, ctx):
    nc = tc.nc
    f32, bf16, f16 = dt.float32, dt.bfloat16, dt.float16

    import os
    debug = bool(int(os.environ.get("K_DEBUG", "0")))
    xT = nc.dram_tensor("xT", [B, C, T], bf16, kind="ExternalInput").ap()
    wqkv = nc.dram_tensor("wqkv", [C, 384], bf16, kind="ExternalInput").ap()
    bqkv = nc.dram_tensor("bqkv", [128, 3], f32, kind="ExternalInput").ap()
    wp = nc.dram_tensor("wp", [128, C], bf16, kind="ExternalInput").ap()
    tri = nc.dram_tensor("tri", [128, 2, 128], bf16, kind="ExternalInput").ap()
    ident = nc.dram_tensor("ident", [128, 128], bf16, kind="ExternalInput").ap()
    outp = nc.dram_tensor("outp", [B, T, C], f16, kind="ExternalOutput").ap()
    if debug:
        dbg_qkv = nc.dram_tensor("dbg_qkv", [3, 128, T], bf16,
                                 kind="ExternalOutput").ap()
        dbg_vt = nc.dram_tensor("dbg_vt", [128, 16, HPC * 65], bf16,
                                kind="ExternalOutput").ap()
        dbg_pt = nc.dram_tensor("dbg_pt", [HPC, 128, QT], bf16,
                                kind="ExternalOutput").ap()
        dbg_un = nc.dram_tensor("dbg_un", [128, QT], bf16,
                                kind="ExternalOutput").ap()
        dbg_ot = nc.dram_tensor("dbg_ot", [HPC, 65, QT], f32,
                                kind="ExternalOutput").ap()

    consts = ctx.enter_context(tc.tile_pool(name="consts", bufs=1))
    xpool = ctx.enter_context(tc.tile_pool(name="xpool", bufs=2))
    qkvpool = ctx.enter_context(tc.tile_pool(name="qkvpool", bufs=6))
    vtmpool = ctx.enter_context(tc.tile_pool(name="vtmpool", bufs=2))
    ptpool = ctx.enter_context(tc.tile_pool(name="ptpool", bufs=3))
    unormp = ctx.enter_context(tc.tile_pool(name="unormp", bufs=2))
    rows = ctx.enter_context(tc.tile_pool(name="rows", bufs=6))
    outsb = ctx.enter_context(tc.tile_pool(name="outsb", bufs=8))
    stp = ctx.enter_context(tc.tile_pool(name="stp", bufs=2, space="PSUM"))
    otp = ctx.enter_context(tc.tile_pool(name="otp", bufs=2, space="PSUM"))
    miscp = ctx.enter_context(tc.tile_pool(name="miscp", bufs=2, space="PSUM"))

    # constants / weights resident in SBUF
    w_sb = consts.tile([128, 8, 384], bf16, tag="w")
    nc.sync.dma_start(out=w_sb, in_=wqkv.rearrange("(k p) f -> p k f", p=128))
    b_sb = consts.tile([128, 3], f32, tag="b")
    nc.sync.dma_start(out=b_sb, in_=bqkv)
    wp_sb = consts.tile([128, C], bf16, tag="wp")
    nc.sync.dma_start(out=wp_sb, in_=wp)
    tri_sb = consts.tile([128, 2, 128], bf16, tag="tri")
    nc.sync.dma_start(out=tri_sb, in_=tri)
    id_sb = consts.tile([128, 128], bf16, tag="id")
    nc.sync.dma_start(out=id_sb, in_=ident)

    # x^T for both batches, in 512-token groups so compute starts early
    xps = []
    for b in range(B):
        xp = xpool.tile([128, 8, T], bf16, tag="xp", name=f"xp{b}")
        xsrc = xT[b].rearrange("(j p) t -> p j t", p=128)
        for tg in range(NQT):
            t0 = tg * QT
            nc.sync.dma_start(out=xp[:, :, t0:t0 + QT],
                              in_=xsrc[:, :, t0:t0 + QT])
        xps.append(xp)

    filler = deque()

    def pop_filler(n=1):
        for _ in range(n):
            if filler:
                filler.popleft()()

    def make_qkv(b):
        """qkvT tiles + one thunk per (tg, m): an 8-MM chain + DVE evict."""
        dsts = [qkvpool.tile([128, T], bf16, tag="qkv", name=f"qkv{b}_{m}")
                for m in range(3)]
        thunks = []
        for tg in range(NQT):
            for m in range(3):
                def th(m=m, tg=tg):
                    t0 = tg * QT
                    pg = miscp.tile([128, QT], f32, tag="misc", name="pg")
                    for kc in range(8):
                        nc.tensor.matmul(
                            pg[:, :],
                            w_sb[:, kc, 128 * m:128 * m + 128],
                            xps[b][:, kc, t0:t0 + QT],
                            start=(kc == 0), stop=(kc == 7),
                        )
                    nc.vector.tensor_scalar_add(
                        dsts[m][:, t0:t0 + QT], pg[:, :], b_sb[:, m:m + 1])
                thunks.append(th)
        return dsts, thunks

    def make_vt(b, vT_t):
        """V to token-major [128, 16, 2*65] with ones columns; 9 thunks."""
        vt = vtmpool.tile([128, 16, HPC * 65], bf16, tag="vtm", name=f"vt{b}")

        def th0():
            nc.vector.memset(
                vt.rearrange("p k (h c) -> p k h c", h=HPC)[:, :, :, 64:65],
                1.0)
        thunks = [th0]
        for j0 in range(0, T // 128, 2):
            def th(j0=j0):
                for j in (j0, j0 + 1):
                    tp = miscp.tile([128, 128], bf16, tag="misc", name="tp")
                    nc.tensor.transpose(
                        tp[:, :], vT_t[:, 128 * j:128 * j + 128], id_sb[:, :])
                    nc.vector.tensor_copy(
                        out=vt[:, j, :].rearrange(
                            "p (h c) -> p h c", h=HPC)[:, :, 0:64],
                        in_=tp.rearrange("p (h c) -> p h c", h=HPC),
                    )
            thunks.append(th)
        return vt, thunks

    def make_proj(b, q0, un):
        """Projection of one q-tile: 8 thunks (1 MM + evict + DMA each)."""
        thunks = []
        for ts in range(QT // 128):
            for ct in range(C // 512):
                def th(ts=ts, ct=ct):
                    a0 = q0 + ts * 128
                    pp = miscp.tile([128, 512], f32, tag="misc", name="pp")
                    nc.tensor.matmul(
                        pp[:, :],
                        un[:, ts * 128:(ts + 1) * 128],
                        wp_sb[:, ct * 512:(ct + 1) * 512],
                        start=True, stop=True,
                    )
                    ob = outsb.tile([128, 512], f16, tag="osb")
                    if (ts + ct) % 2 == 0:
                        nc.scalar.copy(ob[:, :], pp[:, :])
                    else:
                        nc.vector.tensor_copy(ob[:, :], pp[:, :])
                    nc.sync.dma_start(
                        out=outp[b, a0:a0 + 128, ct * 512:(ct + 1) * 512],
                        in_=ob[:, :])
                thunks.append(th)
        return thunks

    # front work: batch-0 QKV for tokens 0..511 and V-chunks 0..3 run
    # densely right away; the rest becomes attention filler.
    qkv0, th_qkv0 = make_qkv(0)
    vt0, th_vt0 = make_vt(0, qkv0[2])
    for i in range(3):       # tg0 x (q, k, v)
        th_qkv0[i]()
    th_vt0[0]()              # ones memset
    th_vt0[1]()              # chunks 0-1
    th_vt0[2]()              # chunks 2-3
    filler.extend(th_qkv0[3:])
    filler.extend(th_vt0[3:])

    qkv1, th_qkv1 = make_qkv(1)
    vt1, th_vt1 = make_vt(1, qkv1[2])
    filler.extend(th_qkv1)
    filler.extend(th_vt1)

    qkv_t, vt_t = {0: qkv0, 1: qkv1}, {0: vt0, 1: vt1}

    for b in range(B):
        qT_t, kT_t, vT_t = qkv_t[b]
        vt = vt_t[b]

        for qt in range(NQT):
            q0 = qt * QT
            nkc = (q0 + QT) // KC
            un = unormp.tile([128, QT], bf16, tag="un", name=f"un{b}{qt}")
            ots = [otp.tile([65, QT], f32, tag="ot", name=f"ot{h}")
                   for h in range(HPC)]

            def emit_o(kc, pt):
                ls = max(0, kc * KC - q0)
                for h in range(HPC):
                    nc.tensor.matmul(
                        ots[h][:, ls:QT],
                        vt[:, kc, 65 * h:65 * h + 65],
                        pt[:, h, ls:QT],
                        start=(kc == 0), stop=(kc == nkc - 1),
                    )

            prev = None
            for kc in range(nkc):
                k0 = kc * KC
                ls = max(0, k0 - q0)
                # supertile: one pool slot holding both heads' S^T in two
                # adjacent PSUM banks -> both matmuls become ready at the
                # same instant and issue back-to-back, running CONCURRENT
                # in different 64-row groups of the PE array.
                st = stp.tile([128, HPC, QT], f32, tag="st", name="st")
                pt = ptpool.tile([128, HPC, QT], bf16, tag="pt", name="pt")
                for h in range(HPC):
                    nc.tensor.matmul(
                        st[:, h, ls:QT],
                        kT_t[64 * h:64 * h + 64, k0:k0 + KC],
                        qT_t[64 * h:64 * h + 64, q0 + ls:q0 + QT],
                        start=True, stop=True,
                    )
                # one wide exp over both heads (3D AP spans both banks)
                nc.scalar.activation(
                    pt[:, :, ls:QT], st[:, :, ls:QT], AF.Exp, scale=SCALE)
                if k0 >= q0:  # diagonal chunk: zero invalid triangle
                    nc.vector.tensor_mul(
                        pt[:, :, ls:ls + 128], pt[:, :, ls:ls + 128],
                        tri_sb[:, :, :])
                if debug and b == 0 and qt == 0 and kc == 1:
                    for h in range(HPC):
                        nc.sync.dma_start(out=dbg_pt[h], in_=pt[:, h, :])
                if prev is not None:
                    emit_o(*prev)
                pop_filler()
                prev = (kc, pt)
            emit_o(*prev)

            # normalize both heads into un
            for h in range(HPC):
                if debug and b == 0 and qt == 0:
                    osb_d = rows.tile([65, QT], f32, tag="osbd",
                                      name=f"osbd{h}")
                    nc.vector.tensor_copy(osb_d[:, :], ots[h][:, :])
                    nc.sync.dma_start(out=dbg_ot[h], in_=osb_d)
                se = rows.tile([1, QT], f32, tag="se", name=f"se{h}")
                nc.vector.tensor_copy(se[:, :], ots[h][64:65, :])
                rc = rows.tile([1, QT], f32, tag="rc", name=f"rc{h}")
                nc.vector.reciprocal_approx_fast(rc[:, :], se[:, :])
                rb = rows.tile([64, QT], f32, tag="rb", name=f"rb{h}")
                nc.gpsimd.partition_broadcast(rb[:, :], rc[:, :])
                nc.vector.tensor_mul(
                    un[64 * h:64 * h + 64, :], ots[h][0:64, :], rb[:, :])
            if debug and b == 0 and qt == 0:
                nc.sync.dma_start(out=dbg_un, in_=un)
            filler.extend(make_proj(b, q0, un))
            pop_filler(3)

    while filler:
        pop_filler()

    if debug:
        for m in range(3):
            nc.sync.dma_start(out=dbg_qkv[m], in_=qkv0[m])
        nc.sync.dma_start(out=dbg_vt, in_=vt0)


def build():
    if "nc" in _CACHE:
        return _CACHE["nc"]
    nc = bacc.Bacc("TRN2", target_bir_lowering=False, debug=False,
                   num_devices=NCORES)
    with tile.TileContext(nc) as tc:
        _emit(tc)
    nc.compile()
    _CACHE["nc"] = nc
    return nc


def make_in_maps(x, qkv_w, qkv_b, proj_w):
    import ml_dtypes
    bf16 = ml_dtypes.bfloat16
    x = np.asarray(x, dtype=np.float32)
    qkv_w = np.asarray(qkv_w, dtype=np.float32)
    qkv_b = np.asarray(qkv_b, dtype=np.float32)
    proj_w = np.asarray(proj_w, dtype=np.float32)

    xT = np.ascontiguousarray(x.transpose(0, 2, 1)).astype(bf16)
    tri1 = (np.arange(128)[None, :] >= np.arange(128)[:, None]).astype(bf16)
    tri = np.ascontiguousarray(np.stack([tri1, tri1], axis=1))
    ident = np.eye(128, dtype=bf16)

    in_maps = []
    for c in range(NCORES):
        s = 64 * HPC * c  # first feature row of this core's heads
        wq = qkv_w[:, s:s + 128]
        wk = qkv_w[:, C + s:C + s + 128]
        wv = qkv_w[:, 2 * C + s:2 * C + s + 128]
        wqkv_c = np.ascontiguousarray(
            np.concatenate([wq, wk, wv], axis=1)).astype(bf16)
        bqkv_c = np.ascontiguousarray(np.stack(
            [qkv_b[s:s + 128], qkv_b[C + s:C + s + 128],
             qkv_b[2 * C + s:2 * C + s + 128]], axis=1))
        wp_c = np.ascontiguousarray(proj_w[s:s + 128, :]).astype(bf16)
        in_maps.append({
            "xT": xT, "wqkv": wqkv_c, "bqkv": bqkv_c, "wp": wp_c,
            "tri": tri, "ident": ident,
        })
    return in_maps


def kernel(x, qkv_w, qkv_b, proj_w, proj_b, _trace=False):
    nc = build()
    in_maps = make_in_maps(x, qkv_w, qkv_b, proj_w)
    res = run_bass_kernel_spmd(nc, in_maps, core_ids=list(range(NCORES)),
                               trace=_trace)
    acc = np.zeros((B, T, C), dtype=np.float64)
    for c in range(NCORES):
        acc += np.asarray(res.results[c]["outp"], dtype=np.float64)
    acc += np.asarray(proj_b, dtype=np.float64)
    out = acc.astype(np.float32)
    _CACHE["last_results"] = res
    return out
